# revision 17
# baseline (speedup 1.0000x reference)
"""GQA attention (B=1, S=2048, D=2048, H=32, KV=4, HD=64) on 8 TRN2 NeuronCores.

Sharding: tensor-parallel over heads. Core c owns q-heads [4c, 4c+4) and kv-head
c//2 (all four q-heads of a core share one kv head). Each core:
  1. Streams x (full) in S-chunks via batched SWDGE cast-DMA (f32->bf16),
     PE-transposes to x^T, and projects Q^T / [K^T; V^T] (weights stationary).
  2. Applies RoPE to Q^T/K^T (head-dim components pre-permuted to
     [evens | odds] in the weight columns so rotation works on contiguous
     32-partition slices; q/k dot products are invariant to the permutation).
  3. Flash-style causal attention per (head, 512-query-chunk): scores^T via
     PE (two heads packed in PE row groups), exp on ScalarE over 1024-wide
     tiles (no max-subtraction; scores are O(5), exact in fp32),
     block-causal masking via precomputed mask tiles, PV matmul with a
     ones-column on V so softmax denominators fall out of the same matmul.
  4. Normalizes (batched reciprocal), AllGathers attn^T per query-chunk
     (overlapped with later attention), and computes its 256-column slice
     of the output projection per gathered chunk.
Returns out^T [256, 2048] per core; the host transposes/concatenates.
"""

import numpy as np

import concourse.bass as bass
import concourse.mybir as mybir
import concourse.tile as tile
from concourse import bacc
from concourse import bass_utils
from concourse.masks import make_identity

F32 = mybir.dt.float32
BF16 = mybir.dt.bfloat16
AF = mybir.ActivationFunctionType
ALU = mybir.AluOpType

S = 2048
D = 2048
HD = 64
NH_LOCAL = 4
CORES = 8
SC = 512
NSC = S // SC
NDC = D // 128
NSB = S // 128

_NC_CACHE = {}


def _dram3(t, row0, nrow_p, nblk, blk_stride, ncol, col0=0):
    """AP over DRAM tensor/AP t: [partition p, block b, col f] ->
    t[row0 + b*blk_stride + p, col0 + f], p<nrow_p, b<nblk, f<ncol."""
    if isinstance(t, bass.AP):
        handle, off0, row_pitch = t.tensor, t.offset, t.tensor.shape[1]
    else:
        handle, off0, row_pitch = t, 0, t.shape[1]
    return bass.AP(
        handle,
        off0 + row0 * row_pitch + col0,
        [[row_pitch, nrow_p], [blk_stride * row_pitch, nblk], [1, ncol]],
    )


def build():
    if "nc" in _NC_CACHE:
        return _NC_CACHE["nc"]
    nc = bacc.Bacc(None, target_bir_lowering=False, debug=False)

    x = nc.declare_dram_parameter("x", [S, D], F32, isOutput=False)
    wq = nc.declare_dram_parameter("wq", [D, 256], F32, isOutput=False)
    wkv = nc.declare_dram_parameter("wkv", [D, 128], F32, isOutput=False)
    wo = nc.declare_dram_parameter("wo", [D, 256], F32, isOutput=False)
    cs = nc.declare_dram_parameter("cs", [32, 2 * S], F32, isOutput=False)
    out = nc.declare_dram_parameter("out", [256, S], F32, isOutput=True)

    with tile.TileContext(nc) as tc:
        with (
            tc.tile_pool(name="const", bufs=1) as const,
            tc.tile_pool(name="wpool", bufs=1) as wpool,
            tc.tile_pool(name="big", bufs=1) as big,
            tc.tile_pool(name="dram", bufs=1, space="DRAM") as dram,
        ):
            # ---- critical path first: identity, x chunk 0, projection weights
            idb = const.tile([128, 128], BF16)
            make_identity(nc, idb[:])
            xbf0 = const.tile([128, 4 * D], BF16)
            for rb in range(4):
                nc.gpsimd.dma_start(
                    out=xbf0[:, rb * D : rb * D + D],
                    in_=x[rb * 128 : rb * 128 + 128, :],
                )
            wqb = wpool.tile([128, NDC * 256], BF16)
            wkvb = wpool.tile([128, NDC * 128], BF16)
            nc.gpsimd.dma_start(out=wqb[:], in_=_dram3(wq, 0, 128, NDC, 128, 256))
            nc.gpsimd.dma_start(out=wkvb[:], in_=_dram3(wkv, 0, 128, NDC, 128, 128))

            # ---- remaining constants ----
            ones64 = const.tile([1, 64], BF16)
            nc.vector.memset(ones64[:], 1.0)
            # 4 diag-mask variants: cols<t*128 -> 0, 128-wide triangle at t*128,
            # cols>=(t+1)*128 -> 1
            dmask = const.tile([128, 4 * SC], BF16)
            nc.gpsimd.memset(dmask[:], 1.0)
            for t in range(4):
                base = t * SC
                if t > 0:
                    nc.gpsimd.memset(dmask[:, base : base + t * 128], 0.0)
                tri = dmask[:, base + t * 128 : base + t * 128 + 128]
                nc.gpsimd.memset(tri, 0.0)
                nc.gpsimd.affine_select(
                    out=tri,
                    in_=tri,
                    compare_op=ALU.is_gt,
                    fill=1.0,
                    base=0,
                    pattern=[[-1, 128]],
                    channel_multiplier=1,
                )
            # cos/sin on 4 partition bands: rows b..b+32: cols [0,S)=cos^T,
            # [S,2S)=sin^T
            cs4 = const.tile([128, 2 * S], BF16)
            for b in range(4):
                nc.gpsimd.dma_start(out=cs4[32 * b : 32 * b + 32, :], in_=cs[:, :])

            wob = wpool.tile([128, NDC * 256], BF16)

            # ---- persistent activations ----
            QT = [big.tile([128, S], BF16, name=f"QT{i}") for i in range(2)]
            KT2 = big.tile([128, S], BF16)
            Vext = big.tile([128, NSB * 65], BF16)
            nc.vector.memset(Vext[:], 1.0)
            AT = [big.tile([128, S], BF16, name=f"AT{i}") for i in range(2)]

            ag_in = [
                dram.tile([256, 2 * SC], BF16, name=f"ag_in{i}") for i in range(2)
            ]
            ag_out = [
                dram.tile(
                    [CORES * 256, 2 * SC],
                    BF16,
                    addr_space="Shared",
                    name=f"ag_out{i}",
                )
                for i in range(2)
            ]

            # ================= phase A: x^T + projections + rope =============
            with (
                tc.tile_pool(name="xp", bufs=2) as xp,
                tc.tile_pool(name="xtp", bufs=2) as xtp,
                tc.tile_pool(name="rp", bufs=3) as rp,
                tc.tile_pool(name="tpps", bufs=2, space="PSUM") as tpps,
                tc.tile_pool(name="ppj", bufs=2, space="PSUM") as ppj,
            ):
                for sc in range(NSC):
                    if sc == 0:
                        xbf = xbf0
                    else:
                        xbf = xp.tile([128, 4 * D], BF16, name="xbf")
                        nc.gpsimd.dma_start(
                            out=xbf[:], in_=_dram3(x, sc * SC, 128, 4, 128, D)
                        )
                    xT = xtp.tile([128, NDC * SC], BF16)
                    for dc in range(NDC):
                        tp = tpps.tile([128, 512], F32)
                        for rb in range(4):
                            nc.tensor.matmul(
                                tp[:, rb * 128 : rb * 128 + 128],
                                xbf[:, rb * D + dc * 128 : rb * D + dc * 128 + 128],
                                idb[:],
                                start=True,
                                stop=True,
                            )
                        nc.scalar.activation(xT[:, dc * SC : dc * SC + 512], tp[:], AF.Copy)

                    def rope_head(psrc, r0, dst, dcol, odd_on_gpsimd):
                        cosb = cs4[r0 : r0 + 32, sc * SC : sc * SC + SC]
                        sinb = cs4[r0 : r0 + 32, S + sc * SC : S + sc * SC + SC]
                        coso = cs4[r0 + 32 : r0 + 64, sc * SC : sc * SC + SC]
                        sino = cs4[r0 + 32 : r0 + 64, S + sc * SC : S + sc * SC + SC]
                        E = psrc[r0 : r0 + 32, :]
                        O = psrc[r0 + 32 : r0 + 64, :]
                        t1 = rp.tile([128, SC], BF16, name="t1")
                        t2 = rp.tile([128, SC], BF16, name="t2")
                        eng = nc.gpsimd if odd_on_gpsimd else nc.vector
                        nc.vector.tensor_mul(t1[r0 : r0 + 32, :], E, cosb)
                        nc.vector.tensor_mul(t2[r0 : r0 + 32, :], O, sino)
                        nc.vector.tensor_sub(
                            dst[r0 : r0 + 32, dcol : dcol + SC],
                            t1[r0 : r0 + 32, :],
                            t2[r0 : r0 + 32, :],
                        )
                        nc.vector.tensor_mul(t1[r0 + 32 : r0 + 64, :], E, sinb)
                        nc.vector.tensor_mul(t2[r0 + 32 : r0 + 64, :], O, coso)
                        eng.tensor_add(
                            dst[r0 + 32 : r0 + 64, dcol : dcol + SC],
                            t1[r0 + 32 : r0 + 64, :],
                            t2[r0 + 32 : r0 + 64, :],
                        )

                    for mb in range(2):
                        psq = ppj.tile([128, SC], F32, name="psq")
                        for dc in range(NDC):
                            nc.tensor.matmul(
                                psq[:],
                                wqb[:, dc * 256 + mb * 128 : dc * 256 + mb * 128 + 128],
                                xT[:, dc * SC : dc * SC + SC],
                                start=(dc == 0),
                                stop=(dc == NDC - 1),
                            )
                        qraw = rp.tile([128, SC], BF16, name="qraw")
                        nc.vector.tensor_copy(qraw[:], psq[:])
                        rope_head(qraw, 0, QT[mb], sc * SC, odd_on_gpsimd=True)
                        rope_head(qraw, 64, QT[mb], sc * SC, odd_on_gpsimd=True)

                    pskv = ppj.tile([128, SC], F32, name="pskv")
                    for dc in range(NDC):
                        nc.tensor.matmul(
                            pskv[:],
                            wkvb[:, dc * 128 : dc * 128 + 128],
                            xT[:, dc * SC : dc * SC + SC],
                            start=(dc == 0),
                            stop=(dc == NDC - 1),
                        )
                    kvraw = rp.tile([128, SC], BF16, name="kvraw")
                    nc.vector.tensor_copy(kvraw[:], pskv[:])
                    rope_head(kvraw, 0, KT2, sc * SC, odd_on_gpsimd=True)
                    nc.gpsimd.tensor_copy(
                        KT2[64:128, sc * SC : sc * SC + SC],
                        KT2[0:64, sc * SC : sc * SC + SC],
                    )
                    tv = tpps.tile([128, 256], F32, name="tv")
                    for jj in range(4):
                        nc.tensor.matmul(
                            tv[:, jj * 64 : jj * 64 + 64],
                            kvraw[64:128, jj * 128 : jj * 128 + 128],
                            idb[64:128, 64:128],
                            start=True,
                            stop=True,
                        )
                    vdst = bass.AP(
                        Vext.tensor,
                        Vext.offset + (sc * 4) * 65,
                        [Vext.ap[0], [65, 4], [1, 64]],
                    )
                    vsrc = bass.AP(tv.tensor, tv.offset, [tv.ap[0], [64, 4], [1, 64]])
                    nc.vector.tensor_copy(vdst, vsrc)

            # wo load deferred to here (not on the startup critical path)
            nc.gpsimd.dma_start(out=wob[:], in_=_dram3(wo, 0, 128, NDC, 128, 256))

            # ================= phase C: attention (+chunked AllGather) =======
            with (
                tc.tile_pool(name="ptp", bufs=3) as ptp,
                tc.tile_pool(name="npool", bufs=5) as npool,
                tc.tile_pool(name="atup", bufs=5) as atup,
                tc.tile_pool(name="spool", bufs=2, space="PSUM") as spool,
                tc.tile_pool(name="pvp", bufs=2, space="PSUM") as pvp,
                tc.tile_pool(name="dbp", bufs=1, space="PSUM") as dbp,
            ):
                for sqc in range(NSC):
                    nblk = 4 * sqc + 4
                    atu = {}
                    rdn = {}
                    for mb in range(2):
                        pv = [
                            pvp.tile([65, SC], F32, name="pv", tag="pv")
                            for _ in range(2)
                        ]
                        for jj in range(0, nblk, 2):
                            nj = min(2, nblk - jj)
                            ps_s = [
                                spool.tile([128, 2 * SC], F32, name="ps_s", tag="ps_s")
                                for _ in range(2)
                            ]
                            for dj in range(nj):
                                j = jj + dj
                                for lh in range(2):
                                    r0 = 64 * lh
                                    nc.tensor.matmul(
                                        ps_s[lh][:, dj * SC : dj * SC + SC],
                                        KT2[r0 : r0 + 64, j * 128 : j * 128 + 128],
                                        QT[mb][r0 : r0 + 64, sqc * SC : sqc * SC + SC],
                                        start=True,
                                        stop=True,
                                    )
                            pt = [
                                ptp.tile([128, 2 * SC], BF16, name="pt", tag="pt")
                                for _ in range(2)
                            ]
                            for lh in range(2):
                                nc.scalar.activation(
                                    pt[lh][:, 0 : nj * SC],
                                    ps_s[lh][:, 0 : nj * SC],
                                    AF.Exp,
                                    scale=0.125,
                                )
                            t0 = jj - 4 * sqc
                            if t0 >= 0:
                                for lh in range(2):
                                    dslc = pt[lh][:, 0 : 2 * SC]
                                    nc.vector.tensor_mul(
                                        dslc,
                                        dslc,
                                        dmask[:, t0 * SC : t0 * SC + 2 * SC],
                                    )
                            for dj in range(nj):
                                j = jj + dj
                                for lh in range(2):
                                    nc.tensor.matmul(
                                        pv[lh][:],
                                        Vext[:, j * 65 : j * 65 + 65],
                                        pt[lh][:, dj * SC : dj * SC + SC],
                                        start=(j == 0),
                                        stop=(j == nblk - 1),
                                    )
                        for lh in range(2):
                            h4 = 2 * mb + lh
                            a = atup.tile([64, SC], BF16, name="atu", tag="atu")
                            nc.vector.tensor_copy(a[:], pv[lh][0:64, :])
                            atu[h4] = a
                            den = npool.tile([1, SC], F32, name="den", tag="den")
                            nc.vector.tensor_copy(den[:], pv[lh][64:65, :])
                            rf = npool.tile([1, SC], F32, name="rf", tag="rf")
                            nc.vector.reciprocal_approx_fast(rf[:], den[:])
                            rb = npool.tile([1, SC], BF16, name="rb", tag="rb")
                            nc.vector.tensor_copy(rb[:], rf[:])
                            rdn[h4] = rb
                    for h4 in range(4):
                        mb, lh = divmod(h4, 2)
                        r0 = 64 * lh
                        dbc = dbp.tile([64, SC], F32, name="dbc")
                        nc.tensor.matmul(
                            dbc[:], ones64[:], rdn[h4][:], start=True, stop=True
                        )
                        dsb = npool.tile([64, SC], BF16, name="dsb", tag="dsb")
                        nc.vector.tensor_copy(dsb[:], dbc[:])
                        nc.vector.tensor_mul(
                            AT[mb][r0 : r0 + 64, sqc * SC : sqc * SC + SC],
                            atu[h4][:],
                            dsb[:],
                        )
                    # chunked AllGather per S-half (after sqc 1 and 3)
                    if sqc % 2 == 1:
                        hh = sqc // 2
                        nc.sync.dma_start(
                            out=ag_in[hh][0:128, :],
                            in_=AT[0][:, hh * 2 * SC : hh * 2 * SC + 2 * SC],
                        )
                        nc.sync.dma_start(
                            out=ag_in[hh][128:256, :],
                            in_=AT[1][:, hh * 2 * SC : hh * 2 * SC + 2 * SC],
                        )
                        nc.gpsimd.collective_compute(
                            "AllGather",
                            ALU.bypass,
                            ins=[ag_in[hh].opt()],
                            outs=[ag_out[hh].opt()],
                            replica_groups=[list(range(CORES))],
                        )

            # ================= phase E: output projection ====================
            with (
                tc.tile_pool(name="agt", bufs=6) as agtp,
                tc.tile_pool(name="otp", bufs=2) as otp,
                tc.tile_pool(name="pop", bufs=2, space="PSUM") as pop,
            ):
                NEC = (CORES * 256) // 128
                for sc2 in range(NSC):
                    ats = []
                    for e4 in range(NEC // 4):
                        t = agtp.tile([128, 4 * SC], BF16, name="agt4", tag="agt4")
                        nc.sync.dma_start(
                            out=t[:],
                            in_=_dram3(
                                ag_out[sc2 // 2],
                                e4 * 512,
                                128,
                                4,
                                128,
                                SC,
                                col0=(sc2 % 2) * SC,
                            ),
                        )
                        ats.append(t)
                    for mb in range(2):
                        po = pop.tile([128, SC], F32, name="po")
                        for e in range(NEC):
                            nc.tensor.matmul(
                                po[:],
                                wob[:, e * 256 + mb * 128 : e * 256 + mb * 128 + 128],
                                ats[e // 4][:, (e % 4) * SC : (e % 4) * SC + SC],
                                start=(e == 0),
                                stop=(e == NEC - 1),
                            )
                        oT = otp.tile([128, SC], F32, name="oT")
                        nc.vector.tensor_copy(oT[:], po[:])
                        nc.sync.dma_start(
                            out=out[mb * 128 : mb * 128 + 128, sc2 * SC : sc2 * SC + SC],
                            in_=oT[:],
                        )

    nc.compile()
    _NC_CACHE["nc"] = nc
    return nc


_PERM = np.concatenate([np.arange(0, HD, 2), np.arange(1, HD, 2)])


def _shard_inputs(x, freqs_cos, freqs_sin, mask, wq, wk, wv, wo):
    x2 = np.ascontiguousarray(x.reshape(S, D), dtype=np.float32)
    cs = np.ascontiguousarray(
        np.concatenate([freqs_cos.T, freqs_sin.T], axis=1), dtype=np.float32
    )
    in_maps = []
    for c in range(CORES):
        g = c // 2
        wq_c = wq[:, 256 * c : 256 * c + 256].reshape(D, NH_LOCAL, HD)[:, :, _PERM]
        wq_c = np.ascontiguousarray(wq_c.reshape(D, 256), dtype=np.float32)
        wk_g = wk[:, HD * g : HD * g + HD][:, _PERM]
        wkv_c = np.ascontiguousarray(
            np.concatenate([wk_g, wv[:, HD * g : HD * g + HD]], axis=1),
            dtype=np.float32,
        )
        wo_c = np.ascontiguousarray(wo[:, 256 * c : 256 * c + 256], dtype=np.float32)
        in_maps.append({"x": x2, "wq": wq_c, "wkv": wkv_c, "wo": wo_c, "cs": cs})
    return in_maps


def kernel(x, freqs_cos, freqs_sin, mask, wq, wk, wv, wo, _trace=False):
    nc = build()
    in_maps = _shard_inputs(x, freqs_cos, freqs_sin, mask, wq, wk, wv, wo)
    res = bass_utils.run_bass_kernel_spmd(
        nc, in_maps, core_ids=list(range(CORES)), trace=_trace
    )
    outp = np.empty((S, D), dtype=np.float32)
    for c in range(CORES):
        outp[:, 256 * c : 256 * c + 256] = res.results[c]["out"].T
    if _trace:
        kernel._last_exec_time_ns = res.exec_time_ns
        kernel._last_results = res
    return outp.reshape(1, S, D)
